# revision 1
# baseline (speedup 1.0000x reference)
"""Trainium2 Bass kernel for nn_CBL_1632087573343 (boundary context loss).

Data-parallel over batch: 8 images -> 8 NeuronCores, one image per core.

Per-core algorithm (reproduces reference._context_loss for one image):
  - er image stored as bf16 [c-chunk(128) x 2, 66*128] flat slabs,
    processed in 2 row-halves; a 1-element-shifted copy (xodd) keeps the
    DVE tensor_tensor multiplies 4B-aligned (2x perf mode) for odd offsets.
  - For each of 12 canonical shifts s (+- pairs folded via weight
    W_s = valid + valid(.+s)) plus the norm pass (s=0):
      DVE: prod_c = er_c * er_c_shifted   (bf16, flat offset dy*128+dx)
      PE:  channel reduction via matmuls with ONE-HOT-COLUMN stationaries:
           block b (512 pixels) uses stationary = window view of a
           [128, 128+NB] tile whose only nonzero column selects output
           partition b; all blocks accumulate into one PSUM [128, 512]
           bank, so 16 blocks x 2 chunks of a half land as rows 0..15.
           Moving operand is the product (N=512 @ 2.4 GHz, LDW hidden).
      ACT: copy psum[0:16, :] -> st [16, 512]
      DMA: fan st out to the dot field tile [y=128, 2|128|2] ([y, x]).
  - Pointwise on [128 y, 132] tiles: dy handled by DMA-shifted copies of
    rn/seg/valid (engines cannot start at partition 1/2), dx by free-dim
    offsets.  cos = dot*rn*rn_s, d = cos - (seg==seg_s), A += d*d*W_s.
  - Reduce A / valid / gt_b; assemble per-image (loss_num, include).
Host combines: loss = sum(loss_num) / max(sum(include), 1).
"""

import sys

sys.path.insert(0, "/opt/trn_rl_repo")

import numpy as np

import concourse.bass as bass
import concourse.tile as tile
from concourse import bacc, mybir

DT = mybir.dt
F32 = DT.float32
BF16 = DT.bfloat16
I32 = DT.int32
ALU = mybir.AluOpType
ACTF = mybir.ActivationFunctionType
AX = mybir.AxisListType

B, C, H, W = 8, 256, 128, 128
HH = 64                          # rows per half
SLAB_ROWS = HH + 2               # rows resident per half (dy<=2 read-ahead)
L_SLAB = 8512                    # >= 66*128+4, padded to a 128B multiple
L_RED = HH * W                   # 8192 columns reduced per (half, shift)
NB = 16                          # 512-pixel blocks per (half, shift)
FX = 192                         # field tile free size (128B-aligned): 2 | 128 x | pad
FOFF = 2                         # x offset inside field tiles

# canonical half of the 24-shift set; even-dx first so the odd-dx slab copy
# (single-buffered) can load while even shifts compute
SHIFTS = [(0, 2), (1, -2), (1, 0), (1, 2), (2, -2), (2, 0), (2, 2),
          (0, 1), (1, -1), (1, 1), (2, -1), (2, 1)]


def _ap(t, offset, dims):
    return bass.AP(t.tensor, offset, [list(d) for d in dims])


def build_kernel(nc):
    er_d = nc.dram_tensor("er", [C, H, W], F32, kind="ExternalInput")
    seg_d = nc.dram_tensor("seg", [H, W], I32, kind="ExternalInput")
    gtb_d = nc.dram_tensor("gtb", [H, W], I32, kind="ExternalInput")
    out_d = nc.dram_tensor("out", [1, 2], F32, kind="ExternalOutput")

    with tile.TileContext(nc) as tc:
        _build(tc, er_d, seg_d, gtb_d, out_d)
    nc.compile()
    return nc


def _build(tc, er_d, seg_d, gtb_d, out_d):
    nc = tc.nc
    from contextlib import ExitStack

    with ExitStack() as ctx:
        const_p = ctx.enter_context(tc.tile_pool(name="const", bufs=1))
        er_p = ctx.enter_context(tc.tile_pool(name="erp", bufs=2))
        xo_p = ctx.enter_context(tc.tile_pool(name="xop", bufs=1))
        prod_p = ctx.enter_context(tc.tile_pool(name="prodp", bufs=2))
        field_p = ctx.enter_context(tc.tile_pool(name="fieldp", bufs=1))
        st_p = ctx.enter_context(tc.tile_pool(name="stp", bufs=3))
        scr_p = ctx.enter_context(tc.tile_pool(name="scrp", bufs=1))
        psum_p = ctx.enter_context(
            tc.tile_pool(name="psump", bufs=3, space="PSUM"))

        ones_f = const_p.tile([128, 32], F32, name="ones_f", tag="ones_f")
        nc.vector.memset(ones_f[:], 1.0)
        # one-hot column bank: sel[:, 128+NB-1-b : 256+NB-1-b] has its only
        # nonzero (ones) column at position b
        SELW = 320
        sel = const_p.tile([128, SELW], BF16, name="sel", tag="sel")
        nc.gpsimd.memset(sel[:], 0.0)
        nc.vector.memset(sel[:, 128 + NB - 1:128 + NB], 1.0)

        P0 = 128 + NB - 1   # absolute position of the ones column

        def sel_view(b):
            # b+1 columns ending at the ones column: output rows 0..b,
            # row b = column sums. Short stationary keeps LDWEIGHTS tiny.
            return sel[:, P0 - b:P0 + 1]

        # ---- label fields ([y, x] layout) ------------------------------
        segi = field_p.tile([H, FX], I32, name="segi", tag="segi")
        nc.gpsimd.memset(segi[:], 0)
        nc.sync.dma_start(out=segi[:, FOFF:FOFF + W], in_=seg_d.ap())
        gtbi = field_p.tile([H, FX], I32, name="gtbi", tag="gtbi")
        nc.gpsimd.memset(gtbi[:], 0)
        nc.sync.dma_start(out=gtbi[:, FOFF:FOFF + W], in_=gtb_d.ap())

        segb = scr_p.tile([H, FX], BF16, name="segb", tag="segb")
        nc.vector.tensor_copy(segb[:], segi[:])
        gtbb = scr_p.tile([H, FX], BF16, name="gtbb", tag="gtbb")
        nc.vector.tensor_copy(gtbb[:], gtbi[:])
        gt_b = field_p.tile([H, FX], BF16, name="gt_b", tag="gt_b")
        nc.vector.tensor_tensor(gt_b[:], segb[:], gtbb[:], op=ALU.mult)

        # interior: x (free col) in [FOFF+2, FOFF+126), y (part) in [2,126)
        iox = scr_p.tile([H, FX], I32, name="iox", tag="iox")
        nc.gpsimd.iota(iox[:], [[1, FX]], channel_multiplier=0)
        xm0 = scr_p.tile([H, FX], BF16, name="xm0", tag="xm0")
        nc.vector.tensor_scalar(xm0[:], iox[:], FOFF + 2, None, op0=ALU.is_ge)
        xm1 = scr_p.tile([H, FX], BF16, name="xm1", tag="xm1")
        nc.vector.tensor_scalar(xm1[:], iox[:], FOFF + 126, None,
                                op0=ALU.is_lt)
        ioy = scr_p.tile([H, 32], I32, name="ioy", tag="ioy")
        nc.gpsimd.iota(ioy[:, 0:1], [[1, 1]], channel_multiplier=1)
        ym0 = scr_p.tile([H, 32], F32, name="ym0", tag="ym0")
        nc.vector.tensor_scalar(ym0[:, 0:1], ioy[:, 0:1], 2, None, op0=ALU.is_ge)
        ym1 = scr_p.tile([H, 32], F32, name="ym1", tag="ym1")
        nc.vector.tensor_scalar(ym1[:, 0:1], ioy[:, 0:1], 126, None, op0=ALU.is_lt)
        ym = scr_p.tile([H, 32], F32, name="ym", tag="ym")
        nc.vector.tensor_tensor(ym[:, 0:1], ym0[:, 0:1], ym1[:, 0:1], op=ALU.mult)

        valid = field_p.tile([H, FX], BF16, name="valid", tag="valid")
        nc.vector.tensor_tensor(valid[:], gt_b[:], xm0[:], op=ALU.mult)
        nc.vector.tensor_tensor(valid[:], valid[:], xm1[:], op=ALU.mult)
        nc.vector.tensor_scalar(valid[:], valid[:], ym[:, 0:1], None, op0=ALU.mult)

        R = scr_p.tile([128, 32], F32, name="R", tag="R")
        nc.vector.memset(R[:], 0.0)
        nc.vector.tensor_reduce(R[:, 1:2], valid[:], axis=AX.X, op=ALU.add)
        nc.vector.tensor_reduce(R[:, 2:3], gt_b[:], axis=AX.X, op=ALU.add)

        # ---- dot fields ([y, x]) ---------------------------------------
        fields = {}
        for s in [(0, 0)] + SHIFTS:
            f = field_p.tile([H, FX], F32, name=f"dot_{s[0]}_{s[1]}",
                             tag=f"dot_{s[0]}_{s[1]}")
            nc.gpsimd.memset(f[:], 0.0)
            fields[s] = f

        A = field_p.tile([H, FX], F32, name="accA", tag="accA")
        nc.gpsimd.memset(A[:], 0.0)

        # ---- main per-half loop ----------------------------------------
        for h in range(2):
            r0 = HH * h
            nflat = (SLAB_ROWS if h == 0 else HH) * W
            # issue both er-chunk loads before the xodd loads: the first
            # even-dx multiplies need er only, and queueing xodd first
            # delays er-c1 by a full transfer (~17us DVE stall measured)
            er_ch, xo_ch = [], []
            for c in range(2):
                e = er_p.tile([128, L_SLAB], BF16, name=f"er{c}",
                              tag=f"er{c}")
                nc.gpsimd.memset(e[:, nflat:L_SLAB], 0.0)
                nc.gpsimd.dma_start(
                    out=_ap(e, 0, [[L_SLAB, 128], [1, nflat]]),
                    in_=_ap(er_d.ap(), c * 128 * H * W + r0 * W,
                            [[H * W, 128], [1, nflat]]))
                er_ch.append(e)
            for c in range(2):
                x = xo_p.tile([128, L_SLAB], BF16, name=f"xo{c}",
                              tag=f"xo{c}")
                nodd = min(nflat, H * W - r0 * W - 1)
                nc.gpsimd.memset(x[:, nodd:L_SLAB], 0.0)
                nc.gpsimd.dma_start(
                    out=_ap(x, 0, [[L_SLAB, 128], [1, nodd]]),
                    in_=_ap(er_d.ap(), c * 128 * H * W + r0 * W + 1,
                            [[H * W, 128], [1, nodd]]))
                xo_ch.append(x)

            for s in [(0, 0)] + SHIFTS:
                dy, dx = s
                off = dy * W + dx
                prods = []
                for c in range(2):
                    p = prod_p.tile([128, L_RED], BF16, name=f"prod{c}",
                                    tag=f"prod{c}")
                    if dx % 2 == 0:
                        in1 = er_ch[c][:, off:off + L_RED]
                    else:
                        in1 = xo_ch[c][:, off - 1:off - 1 + L_RED]
                    nc.vector.tensor_tensor(
                        p[:], er_ch[c][:, 0:L_RED], in1, op=ALU.mult)
                    prods.append(p)

                # block b -> psum row b (one-hot stationary); the block's
                # 512 pixels are the strided y-rows {b, b+16, b+32, b+48}
                # so the staging tile fans out with canonical DMAs below.
                ps = psum_p.tile([128, 512], F32, name="ps", tag="ps")
                n_mm = 2 * NB
                j = 0
                # descending b: the first matmul (b=NB-1) covers rows
                # [0:NB] and start=True-initializes them; later partial
                # writes accumulate into initialized rows only.
                for b in reversed(range(NB)):
                    for c in range(2):
                        nc.tensor.matmul(
                            ps[0:b + 1, 0:512], sel_view(b),
                            _ap(prods[c], 128 * b,
                                [[L_RED, 128], [128 * NB, 4], [1, W]]),
                            start=(j == 0), stop=(j == n_mm - 1),
                            skip_group_check=True)
                        j += 1

                st = st_p.tile([NB, 512], F32, name="st", tag="st")
                nc.scalar.copy(st[:], ps[0:NB, 0:512])

                # st[g, 128q + x] = dot(y = 16q + g, x): 4 DMAs, each to
                # 16 contiguous field partitions (pure partition dim0)
                f = fields[s]
                for q in range(4):
                    nc.sync.dma_start(
                        out=_ap(f, (r0 + 16 * q) * FX + FOFF,
                                [[FX, NB], [1, W]]),
                        in_=_ap(st, 128 * q, [[512, NB], [1, W]]))

        # ---- rn = 1 / max(sqrt(n2), eps) -------------------------------
        rn1 = scr_p.tile([H, FX], F32, name="rn1", tag="rn1")
        nc.vector.memset(rn1[:], 0.0)
        nc.scalar.activation(rn1[:], fields[(0, 0)][:], ACTF.Sqrt)
        nc.vector.tensor_scalar(rn1[:], rn1[:], 1e-8, None, op0=ALU.max)
        rn = field_p.tile([H, FX], F32, name="rn", tag="rn")
        nc.vector.reciprocal(rn[:], rn1[:])

        # ---- dy-shifted copies (engines can't start at partition k) ----
        # f_dk[y, x] = f[y + k, x]; tail rows zero.
        shifted = {0: {"rn": rn, "segi": segi, "valid": valid}}
        for k in (1, 2):
            sd = {}
            for nm, src in (("rn", rn), ("segi", segi), ("valid", valid)):
                t = field_p.tile([H, FX], src.dtype, name=f"{nm}_d{k}",
                                 tag=f"{nm}_d{k}")
                nc.gpsimd.memset(t[:], 0)
                nc.sync.dma_start(
                    out=_ap(t, 0, [[FX, H - k], [1, FX]]),
                    in_=_ap(src, k * FX, [[FX, H - k], [1, FX]]))
                sd[nm] = t
            shifted[k] = sd

        # ---- pointwise per shift ---------------------------------------
        for s in SHIFTS:
            dy, dx = s
            b_ = np.s_[:, FOFF:FOFF + W]
            sh = np.s_[:, FOFF + dx:FOFF + dx + W]
            rn_s = shifted[dy]["rn"]
            segi_s = shifted[dy]["segi"]
            valid_s = shifted[dy]["valid"]

            lab = scr_p.tile([H, FX], BF16, name="lab", tag="lab")
            nc.vector.tensor_tensor(lab[b_], segi[b_], segi_s[sh],
                                    op=ALU.is_equal)
            Wt = scr_p.tile([H, FX], BF16, name="Wt", tag="Wt")
            nc.vector.tensor_tensor(Wt[b_], valid[b_], valid_s[sh],
                                    op=ALU.add)
            t1 = scr_p.tile([H, FX], F32, name="t1", tag="t1")
            nc.vector.tensor_tensor(t1[b_], fields[s][b_], rn[b_],
                                    op=ALU.mult)
            cosb = scr_p.tile([H, FX], BF16, name="cosb", tag="cosb")
            nc.vector.tensor_tensor(cosb[b_], t1[b_], rn_s[sh], op=ALU.mult)
            d = scr_p.tile([H, FX], BF16, name="d", tag="d")
            nc.vector.tensor_tensor(d[b_], cosb[b_], lab[b_],
                                    op=ALU.subtract)
            e2 = scr_p.tile([H, FX], BF16, name="e2", tag="e2")
            nc.vector.tensor_tensor(e2[b_], d[b_], d[b_], op=ALU.mult)
            fw = scr_p.tile([H, FX], BF16, name="fw", tag="fw")
            nc.vector.tensor_tensor(fw[b_], e2[b_], Wt[b_], op=ALU.mult)
            nc.vector.tensor_tensor(A[b_], A[b_], fw[b_], op=ALU.add)

        # ---- final reduction -------------------------------------------
        nc.vector.tensor_reduce(R[:, 0:1], A[:], axis=AX.X, op=ALU.add)

        ps2 = psum_p.tile([128, 512], F32, name="ps2", tag="ps")
        nc.tensor.matmul(ps2[0:1, 0:4], ones_f[:, 0:1], R[:, 0:4],
                         start=True, stop=True)
        scal = scr_p.tile([1, 32], F32, name="scal", tag="scal")
        nc.scalar.copy(scal[0:1, 0:4], ps2[0:1, 0:4])
        # scal: 0=S, 1=cnt, 2=gtbsum | 4=include, 5=max(cnt,1), 6=1/max, 7=loss
        nc.vector.tensor_scalar(scal[0:1, 4:5], scal[0:1, 2:3], 0.0, None,
                                op0=ALU.is_gt)
        nc.vector.tensor_scalar(scal[0:1, 5:6], scal[0:1, 1:2], 1.0, None,
                                op0=ALU.max)
        nc.vector.reciprocal(scal[0:1, 6:7], scal[0:1, 5:6])
        nc.vector.tensor_tensor(scal[0:1, 7:8], scal[0:1, 0:1],
                                scal[0:1, 6:7], op=ALU.mult)
        nc.vector.tensor_tensor(scal[0:1, 7:8], scal[0:1, 7:8],
                                scal[0:1, 4:5], op=ALU.mult)
        nc.vector.tensor_scalar(scal[0:1, 7:8], scal[0:1, 7:8],
                                1.0 / 24.0, None, op0=ALU.mult)

        outt = scr_p.tile([1, 32], F32, name="outt", tag="outt")
        nc.vector.tensor_copy(outt[0:1, 0:1], scal[0:1, 7:8])
        nc.vector.tensor_copy(outt[0:1, 1:2], scal[0:1, 4:5])
        nc.sync.dma_start(out=out_d.ap(), in_=outt[0:1, 0:2])


_NC_CACHE = {}


def get_nc():
    if "nc" not in _NC_CACHE:
        nc = bacc.Bacc("TRN2", target_bir_lowering=False, debug=False)
        build_kernel(nc)
        _NC_CACHE["nc"] = nc
    return _NC_CACHE["nc"]


def kernel(er_input, seg_label, gt_boundary_seg):
    er = np.ascontiguousarray(np.asarray(er_input, dtype=np.float32))
    seg = np.ascontiguousarray(np.asarray(seg_label, dtype=np.int32))
    gtb = np.ascontiguousarray(np.asarray(gt_boundary_seg, dtype=np.int32))
    assert er.shape == (B, C, H, W), er.shape

    nc = get_nc()
    from concourse.bass_utils import run_bass_kernel_spmd

    in_maps = [
        {"er": er[i], "seg": seg[i], "gtb": gtb[i]} for i in range(B)
    ]
    res = run_bass_kernel_spmd(nc, in_maps, list(range(B)))
    outs = [res.results[i]["out"] for i in range(B)]
    loss_nums = np.array([o[0, 0] for o in outs], dtype=np.float64)
    incs = np.array([o[0, 1] for o in outs], dtype=np.float64)
    loss = loss_nums.sum() / max(incs.sum(), 1.0)
    return np.float32(loss)



# revision 2
# speedup vs baseline: 1.0567x; 1.0567x over previous
"""Trainium2 Bass kernel for nn_CBL_1632087573343 (boundary context loss).

Data-parallel over batch: 8 images -> 8 NeuronCores, one image per core.

Per-core redesign (vs the DVE-product baseline): all shift-dot products are
computed on the PE as per-row Gram matmuls, eliminating the 52 big DVE
tensor_tensor product passes entirely.

  - er image as 2 channel-chunk slabs ER_c [128ch, 16768] bf16 (flat pixels,
    2-elem head pad; gpsimd DMA casts f32->bf16 in flight).
  - Image row y (psum slot r = y%4): psum[x, 512r + n] += sum_c
    ER_c[:, x-window]^T @ ER_c[:, 392-wide moving window].  Column n holds
    dot(pixel(y,x), flat pixel 128y-2+n); the 13 shift-dots (incl. the norm
    at (0,0)) live on diagonals n = 128dy + x + dx + 2 -- a per-partition
    skew no DMA/engine AP can express (HWDGE rejects non-partition-aligned
    strides), so de-skew runs on gpsimd:
  - Per 16-row supergroup: 4 psum drains -> bf16 staging [128, 16x392]
    (ACT/DVE alternating), then ap_gather (per-16-partition-group indices)
    pulls 20-wide bands around each diagonal, and local_scatter (true
    per-partition indices) picks the 5 in-band entries, writing the field
    tile F2[x, 256G + 16s + (y%16)], s = 5dy+dx+2.
  - Pointwise runs on all 12 canonical shifts at once in [128, 2048] ops:
    W = valid + valid_s, lab = (seg==seg_s), cos = dot*rn*rn_s,
    accum = sum W*(cos-lab)^2 fused via scalar_tensor_tensor accum_out.
    Mask/label planes are built early (during the er load) from xbar-
    transposed [x,y] masks; dx is a partition shift (4 small DMA copies),
    dy a free-dim offset; slot APs use 16-elem contiguous runs.
Host combines: loss = sum(loss_num) / max(sum(include), 1).
"""

import sys

sys.path.insert(0, "/opt/trn_rl_repo")

import numpy as np

import concourse.bass as bass
import concourse.tile as tile
from concourse import bacc, mybir

DT = mybir.dt
F32 = DT.float32
BF16 = DT.bfloat16
I32 = DT.int32
I16 = DT.int16
ALU = mybir.AluOpType
ACTF = mybir.ActivationFunctionType
AX = mybir.AxisListType

B, C, H, W = 8, 256, 128, 128
NW = 392                 # Gram moving-window width
LER = 16768              # er slab free size (bf16), 2-elem head pad
NSG = 8                  # 16-row supergroups
NST = 16 * NW            # staging cols per supergroup (6272)
NPAIR = 480              # gather pair idxs per supergroup (16r x 3dy x 10)
NBAND = 2 * NPAIR        # gathered band elems (960)
FX2 = 2048               # F2 free size: 8 SG x 256
MFX = 144                # mask tile free size
FOFF = 2                 # x offset inside [y,x] mask tiles


def _ap(t, offset, dims):
    return bass.AP(t.tensor, offset, [list(d) for d in dims])


def build_kernel(nc):
    er_d = nc.dram_tensor("er", [C, H, W], F32, kind="ExternalInput")
    seg_d = nc.dram_tensor("seg", [H, W], I32, kind="ExternalInput")
    gtb_d = nc.dram_tensor("gtb", [H, W], I32, kind="ExternalInput")
    gidx_d = nc.dram_tensor("gidx", [128, NPAIR // 16], I16,
                            kind="ExternalInput")
    sidx_d = nc.dram_tensor("sidx", [128, NBAND], I16, kind="ExternalInput")
    out_d = nc.dram_tensor("out", [1, 2], F32, kind="ExternalOutput")

    with tile.TileContext(nc) as tc:
        _build(tc, er_d, seg_d, gtb_d, gidx_d, sidx_d, out_d)
    nc.compile()
    return nc


def _build(tc, er_d, seg_d, gtb_d, gidx_d, sidx_d, out_d):
    nc = tc.nc
    from contextlib import ExitStack

    with ExitStack() as ctx:
        const_p = ctx.enter_context(tc.tile_pool(name="const", bufs=1))
        er_p = ctx.enter_context(tc.tile_pool(name="erp", bufs=1))
        st_p = ctx.enter_context(tc.tile_pool(name="stp", bufs=4))
        scr_p = ctx.enter_context(tc.tile_pool(name="scrp", bufs=1))
        psum_p = ctx.enter_context(
            tc.tile_pool(name="psump", bufs=2, space="PSUM"))

        ones_f = const_p.tile([128, 32], F32, name="ones_f", tag="ones_f")
        nc.vector.memset(ones_f[:], 1.0)

        gidx = const_p.tile([128, NPAIR // 16], I16, name="gidx", tag="gidx")
        nc.sync.dma_start(out=gidx[:], in_=gidx_d.ap())
        sidx = const_p.tile([128, NBAND], I16, name="sidx", tag="sidx")
        nc.sync.dma_start(out=sidx[:], in_=sidx_d.ap())

        # ---- er slabs (start loads first; everything else overlaps) -----
        ers = []
        for c in range(2):
            e = er_p.tile([128, LER], BF16, name=f"er{c}", tag=f"er{c}")
            nc.vector.memset(e[:, 0:2], 0.0)
            nc.vector.memset(e[:, 2 + H * W:LER], 0.0)
            ers.append(e)
        NPIECE = 4
        PC = H * W // NPIECE  # 4096
        for k in range(NPIECE):
            for c in range(2):
                nc.gpsimd.dma_start(
                    out=_ap(ers[c], 2 + k * PC, [[LER, 128], [1, PC]]),
                    in_=_ap(er_d.ap(), c * 128 * H * W + k * PC,
                            [[H * W, 128], [1, PC]]))

        # ---- label fields, [y, x] orientation ---------------------------
        segi = scr_p.tile([H, MFX], I32, name="segi", tag="segi")
        nc.vector.memset(segi[:], 0)
        nc.sync.dma_start(out=segi[:, FOFF:FOFF + W], in_=seg_d.ap())
        gtbi = scr_p.tile([H, MFX], I32, name="gtbi", tag="gtbi")
        nc.vector.memset(gtbi[:], 0)
        nc.sync.dma_start(out=gtbi[:, FOFF:FOFF + W], in_=gtb_d.ap())

        segb = scr_p.tile([H, MFX], BF16, name="segb", tag="segb")
        nc.vector.tensor_copy(segb[:], segi[:])
        gtbb = scr_p.tile([H, MFX], BF16, name="gtbb", tag="gtbb")
        nc.vector.tensor_copy(gtbb[:], gtbi[:])
        gt_b = scr_p.tile([H, MFX], BF16, name="gt_b", tag="gt_b")
        nc.vector.tensor_tensor(gt_b[:], segb[:], gtbb[:], op=ALU.mult)

        # interior: x in [2,126), y in [2,126)
        iox = scr_p.tile([H, MFX], I32, name="iox", tag="iox")
        nc.gpsimd.iota(iox[:], [[1, MFX]], channel_multiplier=0)
        xm0 = scr_p.tile([H, MFX], BF16, name="xm0", tag="xm0")
        nc.vector.tensor_scalar(xm0[:], iox[:], FOFF + 2, None, op0=ALU.is_ge)
        xm1 = scr_p.tile([H, MFX], BF16, name="xm1", tag="xm1")
        nc.vector.tensor_scalar(xm1[:], iox[:], FOFF + 126, None,
                                op0=ALU.is_lt)
        ioy = scr_p.tile([H, 32], I32, name="ioy", tag="ioy")
        nc.gpsimd.iota(ioy[:, 0:1], [[1, 1]], channel_multiplier=1)
        ym0 = scr_p.tile([H, 32], F32, name="ym0", tag="ym0")
        nc.vector.tensor_scalar(ym0[:, 0:1], ioy[:, 0:1], 2, None,
                                op0=ALU.is_ge)
        ym1 = scr_p.tile([H, 32], F32, name="ym1", tag="ym1")
        nc.vector.tensor_scalar(ym1[:, 0:1], ioy[:, 0:1], 126, None,
                                op0=ALU.is_lt)
        ym = scr_p.tile([H, 32], F32, name="ym", tag="ym")
        nc.vector.tensor_tensor(ym[:, 0:1], ym0[:, 0:1], ym1[:, 0:1],
                                op=ALU.mult)

        valid = scr_p.tile([H, MFX], BF16, name="valid", tag="valid")
        nc.vector.tensor_tensor(valid[:], gt_b[:], xm0[:], op=ALU.mult)
        nc.vector.tensor_tensor(valid[:], valid[:], xm1[:], op=ALU.mult)
        nc.vector.tensor_scalar(valid[:], valid[:], ym[:, 0:1], None,
                                op0=ALU.mult)

        R = scr_p.tile([128, 32], F32, name="R", tag="R")
        nc.vector.memset(R[:], 0.0)
        nc.vector.tensor_reduce(R[:, 1:2], valid[:], axis=AX.X, op=ALU.add)
        nc.vector.tensor_reduce(R[:, 2:3], gt_b[:], axis=AX.X, op=ALU.add)

        # ---- transpose valid/seg to [x, y] orientation ------------------
        validT = const_p.tile([128, MFX], BF16, name="validT", tag="validT")
        nc.vector.memset(validT[:], 0.0)
        nc.sync.dma_start_transpose(validT[:, 0:W], valid[:, FOFF:FOFF + W])
        segbT = const_p.tile([128, MFX], BF16, name="segbT", tag="segbT")
        nc.vector.memset(segbT[:], 0.0)
        nc.sync.dma_start_transpose(segbT[:, 0:W], segb[:, FOFF:FOFF + W])

        # ---- dx-shifted copies (partition shifts via DMA) ---------------
        def make_shift(nm, srct, dx):
            t = scr_p.tile([128, MFX], srct.dtype, name=f"{nm}_x{dx}",
                           tag=f"{nm}_x{dx}")
            nc.vector.memset(t[:], 0)
            cnt = 128 - abs(dx)
            if dx > 0:
                nc.sync.dma_start(
                    out=_ap(t, 0, [[MFX, cnt], [1, MFX]]),
                    in_=_ap(srct, dx * MFX, [[MFX, cnt], [1, MFX]]))
            else:
                nc.sync.dma_start(
                    out=_ap(t, -dx * MFX, [[MFX, cnt], [1, MFX]]),
                    in_=_ap(srct, 0, [[MFX, cnt], [1, MFX]]))
            return t

        vsh = {0: validT}
        ssh = {0: segbT}
        for dx in (1, 2, -1, -2):
            vsh[dx] = make_shift("v", validT, dx)
            ssh[dx] = make_shift("s", segbT, dx)

        # slot s = 5*dy + dx + 2; F2 col = 256*(y//16) + 16*s + (y%16)
        def slot_op(out_t, base_t, sh_t, dx, dy, op, eng=None):
            s = 5 * dy + dx + 2
            (eng or nc.vector).tensor_tensor(
                _ap(out_t, 16 * s, [[FX2, 128], [256, 8], [1, 16]]),
                _ap(base_t, 0, [[MFX, 128], [16, 8], [1, 16]]),
                _ap(sh_t, dy, [[MFX, 128], [16, 8], [1, 16]]),
                op=op)

        def slots():
            for dx in (-2, -1, 0, 1, 2):
                for dy in ((0, 1, 2) if dx > 0 else (1, 2)):
                    yield dx, dy

        # W and lab planes are input-only: build during the er load
        Wt = scr_p.tile([128, FX2], BF16, name="Wt", tag="Wt")
        nc.vector.memset(Wt[:], 0.0)
        lab = scr_p.tile([128, FX2], BF16, name="lab", tag="lab")
        nc.vector.memset(lab[:], 0.0)
        for dx, dy in slots():
            slot_op(Wt, validT, vsh[dx], dx, dy, ALU.add)
            slot_op(lab, segbT, ssh[dx], dx, dy, ALU.is_equal)

        # ---- Gram row loop ---------------------------------------------
        # gpsimd custom-op types are batched (gathers inline, scatters at
        # the end): alternating op types reloads Q7 ucode (~16us bubbles).
        f2 = const_p.tile([128, FX2], BF16, name="f2", tag="f2")

        for G in range(NSG):
            st16 = st_p.tile([128, NST], BF16, name="st16", tag="st16")
            for q in range(4):
                P = psum_p.tile([128, 2048], F32, name="P", tag="P")
                for r in range(4):
                    y = 16 * G + 4 * q + r
                    for c in range(2):
                        nc.tensor.matmul(
                            P[:, 512 * r:512 * r + NW],
                            ers[c][:, 2 + 128 * y:2 + 128 * y + 128],
                            ers[c][:, 128 * y:128 * y + NW],
                            start=(c == 0), stop=(c == 1),
                            skip_group_check=True)
                dst = st16[:, 4 * NW * q:4 * NW * (q + 1)]
                src = _ap(P, 0, [[2048, 128], [512, 4], [1, NW]])
                if q % 2 == 0:
                    nc.scalar.copy(dst, src)
                else:
                    nc.vector.tensor_copy(dst, src)
            bands = st_p.tile([128, NBAND], BF16, name="bands", tag="bands")
            nc.gpsimd.ap_gather(
                bands[:], st16[:], gidx[:],
                channels=128, num_elems=NST // 2, d=2, num_idxs=NPAIR)
            nc.gpsimd.local_scatter(
                f2[:, 256 * G:256 * (G + 1)], bands[:], sidx[:],
                channels=128, num_elems=256, num_idxs=NBAND)

        # ---- rn = 1 / max(sqrt(n2), eps), [x, y] ------------------------
        rn1 = scr_p.tile([128, MFX], F32, name="rn1", tag="rn1")
        nc.vector.memset(rn1[:], 0.0)
        nc.scalar.activation(
            _ap(rn1, 0, [[MFX, 128], [16, 8], [1, 16]]),
            _ap(f2, 32, [[FX2, 128], [256, 8], [1, 16]]),
            ACTF.Sqrt)
        nc.vector.tensor_scalar(rn1[:, 0:128], rn1[:, 0:128], 1e-8, None,
                                op0=ALU.max)
        rnT = scr_p.tile([128, MFX], F32, name="rnT", tag="rnT")
        nc.vector.memset(rnT[:], 0.0)
        nc.vector.reciprocal(rnT[:, 0:128], rn1[:, 0:128])

        rsh = {0: rnT}
        for dx in (1, 2, -1, -2):
            rsh[dx] = make_shift("r", rnT, dx)

        pA = scr_p.tile([128, FX2], BF16, name="pA", tag="pA")
        pB = scr_p.tile([128, FX2], BF16, name="pB", tag="pB")
        nc.vector.memset(pA[:], 0.0)
        for dx, dy in slots():
            slot_op(pA, rnT, rsh[dx], dx, dy, ALU.mult)

        # ---- pointwise + fused reduction (A/B ping-pong) ----------------
        nc.vector.tensor_tensor(pB[:], f2[:], pA[:], op=ALU.mult)   # cos
        nc.vector.tensor_tensor(pA[:], pB[:], lab[:], op=ALU.subtract)  # d
        nc.vector.tensor_tensor(pB[:], pA[:], pA[:], op=ALU.mult)   # d^2
        nc.vector.scalar_tensor_tensor(
            pA[:], pB[:], 1.0, Wt[:], op0=ALU.mult, op1=ALU.mult,
            accum_out=R[:, 0:1])

        # ---- final scalar tail ------------------------------------------
        ps2 = psum_p.tile([128, 2048], F32, name="P", tag="P")
        nc.tensor.matmul(ps2[0:1, 0:4], ones_f[:, 0:1], R[:, 0:4],
                         start=True, stop=True)
        scal = scr_p.tile([1, 32], F32, name="scal", tag="scal")
        nc.scalar.copy(scal[0:1, 0:4], ps2[0:1, 0:4])
        # scal: 0=S, 1=cnt, 2=gtbsum | 4=include, 5=max(cnt,1), 6=1/max, 7=loss
        nc.vector.tensor_scalar(scal[0:1, 4:5], scal[0:1, 2:3], 0.0, None,
                                op0=ALU.is_gt)
        nc.vector.tensor_scalar(scal[0:1, 5:6], scal[0:1, 1:2], 1.0, None,
                                op0=ALU.max)
        nc.vector.reciprocal(scal[0:1, 6:7], scal[0:1, 5:6])
        nc.vector.tensor_tensor(scal[0:1, 7:8], scal[0:1, 0:1],
                                scal[0:1, 6:7], op=ALU.mult)
        nc.vector.tensor_tensor(scal[0:1, 7:8], scal[0:1, 7:8],
                                scal[0:1, 4:5], op=ALU.mult)
        nc.vector.tensor_scalar(scal[0:1, 7:8], scal[0:1, 7:8],
                                1.0 / 24.0, None, op0=ALU.mult)

        outt = scr_p.tile([1, 32], F32, name="outt", tag="outt")
        nc.vector.tensor_copy(outt[0:1, 0:1], scal[0:1, 7:8])
        nc.vector.tensor_copy(outt[0:1, 1:2], scal[0:1, 4:5])
        nc.sync.dma_start(out=out_d.ap(), in_=outt[0:1, 0:2])


_NC_CACHE = {}


def _make_idx():
    # gather: per core group g16=(x//16), band (r16, dy):
    #   pairs start (392*r16 + 128*dy + 16*g16)/2, 10 pairs
    gidx = np.zeros((128, NPAIR // 16), np.int16)
    for g16 in range(8):
        lo = 16 * g16
        unwrapped = np.zeros(NPAIR, np.int64)
        i = 0
        for r16 in range(16):
            for dy in range(3):
                start = 392 * r16 + 128 * dy + 16 * g16
                for j in range(10):
                    unwrapped[i] = start // 2 + j
                    i += 1
        for i in range(NPAIR):
            gidx[lo + i % 16, i // 16] = unwrapped[i]

    # scatter: bands elem m: pair i=m//2, o=m%2, band=i//10, jp=i%10,
    # j = 2*jp+o in [0,20); r16 = band//3, dy = band%3;
    # dx = j - (x%16) - 2; target 16*(5dy+dx+2) + r16 if |dx|<=2
    sidx = np.full((128, NBAND), -1, np.int16)
    for x in range(128):
        xm = x % 16
        for i in range(NPAIR):
            band, jp = divmod(i, 10)
            r16, dy = divmod(band, 3)
            for o in range(2):
                j = 2 * jp + o
                dx = j - xm - 2
                if -2 <= dx <= 2:
                    sidx[x, 2 * i + o] = 16 * (5 * dy + dx + 2) + r16
    return gidx, sidx


def get_nc():
    if "nc" not in _NC_CACHE:
        nc = bacc.Bacc("TRN2", target_bir_lowering=False, debug=False)
        build_kernel(nc)
        _NC_CACHE["nc"] = nc
    return _NC_CACHE["nc"]


def kernel(er_input, seg_label, gt_boundary_seg):
    er = np.ascontiguousarray(np.asarray(er_input, dtype=np.float32))
    seg = np.ascontiguousarray(np.asarray(seg_label, dtype=np.int32))
    gtb = np.ascontiguousarray(np.asarray(gt_boundary_seg, dtype=np.int32))
    assert er.shape == (B, C, H, W), er.shape

    nc = get_nc()
    from concourse.bass_utils import run_bass_kernel_spmd

    gidx, sidx = _make_idx()
    in_maps = [
        {"er": er[i], "seg": seg[i], "gtb": gtb[i], "gidx": gidx,
         "sidx": sidx}
        for i in range(B)
    ]
    res = run_bass_kernel_spmd(nc, in_maps, list(range(B)))
    outs = [res.results[i]["out"] for i in range(B)]
    loss_nums = np.array([o[0, 0] for o in outs], dtype=np.float64)
    incs = np.array([o[0, 1] for o in outs], dtype=np.float64)
    loss = loss_nums.sum() / max(incs.sum(), 1.0)
    return np.float32(loss)


# revision 3
# speedup vs baseline: 1.0616x; 1.0046x over previous
"""Trainium2 Bass kernel for nn_CBL_1632087573343 (boundary context loss).

Data-parallel over batch: 8 images -> 8 NeuronCores, one image per core.

Per-core redesign (vs the DVE-product baseline): all shift-dot products are
computed on the PE as per-row Gram matmuls, eliminating the 52 big DVE
tensor_tensor product passes entirely.

  - er image as 2 channel-chunk slabs ER_c [128ch, 16768] bf16 (flat pixels,
    2-elem head pad; gpsimd DMA casts f32->bf16 in flight).
  - Image row y (psum slot r = y%4): psum[x, 512r + n] += sum_c
    ER_c[:, x-window]^T @ ER_c[:, 392-wide moving window].  Column n holds
    dot(pixel(y,x), flat pixel 128y-2+n); the 13 shift-dots (incl. the norm
    at (0,0)) live on diagonals n = 128dy + x + dx + 2 -- a per-partition
    skew no DMA/engine AP can express (HWDGE rejects non-partition-aligned
    strides), so de-skew runs on gpsimd:
  - Per 16-row supergroup: 4 psum drains -> bf16 staging [128, 16x392]
    (ACT/DVE alternating), then ap_gather (per-16-partition-group indices)
    pulls 20-wide bands around each diagonal, and local_scatter (true
    per-partition indices) picks the 5 in-band entries, writing the field
    tile F2[x, 256G + 16s + (y%16)], s = 5dy+dx+2.
  - Pointwise runs on all 12 canonical shifts at once in [128, 2048] ops:
    W = valid + valid_s, lab = (seg==seg_s), cos = dot*rn*rn_s,
    accum = sum W*(cos-lab)^2 fused via scalar_tensor_tensor accum_out.
    Mask/label planes are built early (during the er load) from xbar-
    transposed [x,y] masks; dx is a partition shift (4 small DMA copies),
    dy a free-dim offset; slot APs use 16-elem contiguous runs.
Host combines: loss = sum(loss_num) / max(sum(include), 1).
"""

import sys

sys.path.insert(0, "/opt/trn_rl_repo")

import numpy as np

import concourse.bass as bass
import concourse.tile as tile
from concourse import bacc, mybir

DT = mybir.dt
F32 = DT.float32
BF16 = DT.bfloat16
I32 = DT.int32
I16 = DT.int16
ALU = mybir.AluOpType
ACTF = mybir.ActivationFunctionType
AX = mybir.AxisListType

B, C, H, W = 8, 256, 128, 128
NW = 392                 # Gram moving-window width
LER = 16768              # er slab free size (bf16), 2-elem head pad
NSG = 8                  # 16-row supergroups
NST = 16 * NW            # staging cols per supergroup (6272)
NPAIR = 480              # gather pair idxs per supergroup (16r x 3dy x 10)
NBAND = 2 * NPAIR        # gathered band elems (960)
FX2 = 2048               # F2 free size: 8 SG x 256
MFX = 144                # mask tile free size
FOFF = 2                 # x offset inside [y,x] mask tiles


def _ap(t, offset, dims):
    return bass.AP(t.tensor, offset, [list(d) for d in dims])


def build_kernel(nc):
    er_d = nc.dram_tensor("er", [C, H, W], F32, kind="ExternalInput")
    seg_d = nc.dram_tensor("seg", [H, W], I32, kind="ExternalInput")
    gtb_d = nc.dram_tensor("gtb", [H, W], I32, kind="ExternalInput")
    gidx_d = nc.dram_tensor("gidx", [128, NPAIR // 16], I16,
                            kind="ExternalInput")
    sidx_d = nc.dram_tensor("sidx", [128, NBAND], I16, kind="ExternalInput")
    out_d = nc.dram_tensor("out", [1, 2], F32, kind="ExternalOutput")

    with tile.TileContext(nc) as tc:
        _build(tc, er_d, seg_d, gtb_d, gidx_d, sidx_d, out_d)
    nc.compile()
    return nc


def _build(tc, er_d, seg_d, gtb_d, gidx_d, sidx_d, out_d):
    nc = tc.nc
    from contextlib import ExitStack

    with ExitStack() as ctx:
        const_p = ctx.enter_context(tc.tile_pool(name="const", bufs=1))
        er_p = ctx.enter_context(tc.tile_pool(name="erp", bufs=1))
        st_p = ctx.enter_context(tc.tile_pool(name="stp", bufs=4))
        scr_p = ctx.enter_context(tc.tile_pool(name="scrp", bufs=1))
        psum_p = ctx.enter_context(
            tc.tile_pool(name="psump", bufs=2, space="PSUM"))

        ones_f = const_p.tile([128, 32], F32, name="ones_f", tag="ones_f")
        nc.vector.memset(ones_f[:], 1.0)

        gidx = const_p.tile([128, NPAIR // 16], I16, name="gidx", tag="gidx")
        nc.sync.dma_start(out=gidx[:], in_=gidx_d.ap())
        sidx = const_p.tile([128, NBAND], I16, name="sidx", tag="sidx")
        nc.sync.dma_start(out=sidx[:], in_=sidx_d.ap())

        # ---- er slabs (start loads first; everything else overlaps) -----
        ers = []
        for c in range(2):
            e = er_p.tile([128, LER], BF16, name=f"er{c}", tag=f"er{c}")
            nc.vector.memset(e[:, 0:2], 0.0)
            nc.vector.memset(e[:, 2 + H * W:LER], 0.0)
            ers.append(e)
        NPIECE = 4
        PC = H * W // NPIECE  # 4096
        for k in range(NPIECE):
            for c in range(2):
                nc.gpsimd.dma_start(
                    out=_ap(ers[c], 2 + k * PC, [[LER, 128], [1, PC]]),
                    in_=_ap(er_d.ap(), c * 128 * H * W + k * PC,
                            [[H * W, 128], [1, PC]]))

        # ---- label fields, [y, x] orientation ---------------------------
        segi = scr_p.tile([H, MFX], I32, name="segi", tag="segi")
        nc.vector.memset(segi[:], 0)
        nc.sync.dma_start(out=segi[:, FOFF:FOFF + W], in_=seg_d.ap())
        gtbi = scr_p.tile([H, MFX], I32, name="gtbi", tag="gtbi")
        nc.vector.memset(gtbi[:], 0)
        nc.sync.dma_start(out=gtbi[:, FOFF:FOFF + W], in_=gtb_d.ap())

        segb = scr_p.tile([H, MFX], BF16, name="segb", tag="segb")
        nc.vector.tensor_copy(segb[:], segi[:])
        gtbb = scr_p.tile([H, MFX], BF16, name="gtbb", tag="gtbb")
        nc.vector.tensor_copy(gtbb[:], gtbi[:])
        gt_b = scr_p.tile([H, MFX], BF16, name="gt_b", tag="gt_b")
        nc.vector.tensor_tensor(gt_b[:], segb[:], gtbb[:], op=ALU.mult)

        # interior: x in [2,126), y in [2,126)
        iox = scr_p.tile([H, MFX], I32, name="iox", tag="iox")
        nc.gpsimd.iota(iox[:], [[1, MFX]], channel_multiplier=0)
        xm0 = scr_p.tile([H, MFX], BF16, name="xm0", tag="xm0")
        nc.vector.tensor_scalar(xm0[:], iox[:], FOFF + 2, None, op0=ALU.is_ge)
        xm1 = scr_p.tile([H, MFX], BF16, name="xm1", tag="xm1")
        nc.vector.tensor_scalar(xm1[:], iox[:], FOFF + 126, None,
                                op0=ALU.is_lt)
        ioy = scr_p.tile([H, 32], I32, name="ioy", tag="ioy")
        nc.gpsimd.iota(ioy[:, 0:1], [[1, 1]], channel_multiplier=1)
        ym0 = scr_p.tile([H, 32], F32, name="ym0", tag="ym0")
        nc.vector.tensor_scalar(ym0[:, 0:1], ioy[:, 0:1], 2, None,
                                op0=ALU.is_ge)
        ym1 = scr_p.tile([H, 32], F32, name="ym1", tag="ym1")
        nc.vector.tensor_scalar(ym1[:, 0:1], ioy[:, 0:1], 126, None,
                                op0=ALU.is_lt)
        ym = scr_p.tile([H, 32], F32, name="ym", tag="ym")
        nc.vector.tensor_tensor(ym[:, 0:1], ym0[:, 0:1], ym1[:, 0:1],
                                op=ALU.mult)

        valid = scr_p.tile([H, MFX], BF16, name="valid", tag="valid")
        nc.vector.tensor_tensor(valid[:], gt_b[:], xm0[:], op=ALU.mult)
        nc.vector.tensor_tensor(valid[:], valid[:], xm1[:], op=ALU.mult)
        nc.vector.tensor_scalar(valid[:], valid[:], ym[:, 0:1], None,
                                op0=ALU.mult)

        R = scr_p.tile([128, 32], F32, name="R", tag="R")
        nc.vector.memset(R[:], 0.0)
        nc.vector.tensor_reduce(R[:, 1:2], valid[:], axis=AX.X, op=ALU.add)
        nc.vector.tensor_reduce(R[:, 2:3], gt_b[:], axis=AX.X, op=ALU.add)

        # ---- transpose valid/seg to [x, y] orientation ------------------
        validT = const_p.tile([128, MFX], BF16, name="validT", tag="validT")
        nc.vector.memset(validT[:], 0.0)
        nc.sync.dma_start_transpose(validT[:, 0:W], valid[:, FOFF:FOFF + W])
        segbT = const_p.tile([128, MFX], BF16, name="segbT", tag="segbT")
        nc.vector.memset(segbT[:], 0.0)
        nc.sync.dma_start_transpose(segbT[:, 0:W], segb[:, FOFF:FOFF + W])

        # ---- dx-shifted copies (partition shifts via DMA) ---------------
        def make_shift(nm, srct, dx):
            t = scr_p.tile([128, MFX], srct.dtype, name=f"{nm}_x{dx}",
                           tag=f"{nm}_x{dx}")
            nc.vector.memset(t[:], 0)
            cnt = 128 - abs(dx)
            if dx > 0:
                nc.sync.dma_start(
                    out=_ap(t, 0, [[MFX, cnt], [1, MFX]]),
                    in_=_ap(srct, dx * MFX, [[MFX, cnt], [1, MFX]]))
            else:
                nc.sync.dma_start(
                    out=_ap(t, -dx * MFX, [[MFX, cnt], [1, MFX]]),
                    in_=_ap(srct, 0, [[MFX, cnt], [1, MFX]]))
            return t

        vsh = {0: validT}
        ssh = {0: segbT}
        for dx in (1, 2, -1, -2):
            vsh[dx] = make_shift("v", validT, dx)
            ssh[dx] = make_shift("s", segbT, dx)

        # slot s = 5*dy + dx + 2; F2 col = 256*(y//16) + 16*s + (y%16)
        def slot_op(out_t, base_t, sh_t, dx, dy, op, eng=None):
            s = 5 * dy + dx + 2
            (eng or nc.vector).tensor_tensor(
                _ap(out_t, 16 * s, [[FX2, 128], [256, 8], [1, 16]]),
                _ap(base_t, 0, [[MFX, 128], [16, 8], [1, 16]]),
                _ap(sh_t, dy, [[MFX, 128], [16, 8], [1, 16]]),
                op=op)

        def slots():
            for dx in (-2, -1, 0, 1, 2):
                for dy in ((0, 1, 2) if dx > 0 else (1, 2)):
                    yield dx, dy

        # W and lab planes are input-only: build during the er load
        Wt = scr_p.tile([128, FX2], BF16, name="Wt", tag="Wt")
        nc.vector.memset(Wt[:], 0.0)
        lab = scr_p.tile([128, FX2], BF16, name="lab", tag="lab")
        nc.vector.memset(lab[:], 0.0)
        for dx, dy in slots():
            slot_op(Wt, validT, vsh[dx], dx, dy, ALU.add)
            slot_op(lab, segbT, ssh[dx], dx, dy, ALU.is_equal)

        # ---- Gram row loop ---------------------------------------------
        # gpsimd custom-op types are batched (gathers inline, scatters at
        # the end): alternating op types reloads Q7 ucode (~16us bubbles).
        f2 = const_p.tile([128, FX2], BF16, name="f2", tag="f2")
        from concourse.tile_rust import add_dep_helper

        band_tiles = []
        gather_insts = []
        for G in range(NSG):
            st16 = st_p.tile([128, NST], BF16, name="st16", tag="st16")
            for h in range(8):
                P = psum_p.tile([128, 1024], F32, name="P", tag="P")
                for r2 in range(2):
                    y = 16 * G + 2 * h + r2
                    for c in range(2):
                        nc.tensor.matmul(
                            P[:, 512 * r2:512 * r2 + NW],
                            ers[c][:, 2 + 128 * y:2 + 128 * y + 128],
                            ers[c][:, 128 * y:128 * y + NW],
                            start=(c == 0), stop=(c == 1),
                            skip_group_check=True)
                dst = st16[:, 2 * NW * h:2 * NW * (h + 1)]
                src = _ap(P, 0, [[1024, 128], [512, 2], [1, NW]])
                if h % 2 == 0:
                    nc.scalar.copy(dst, src)
                else:
                    nc.vector.tensor_copy(dst, src)
            bands = const_p.tile([128, NBAND], BF16, name=f"bands{G}",
                                 tag=f"bands{G}")
            gi = nc.gpsimd.ap_gather(
                bands[:], st16[:], gidx[:],
                channels=128, num_elems=NST // 2, d=2, num_idxs=NPAIR)
            band_tiles.append(bands)
            gather_insts.append(gi)

        # force every scatter after the last gather: ap_gather and
        # local_scatter live in different Q7 ucode libraries; interleaving
        # them reloads ucode (~10-20us per switch)
        for G in range(NSG):
            si = nc.gpsimd.local_scatter(
                f2[:, 256 * G:256 * (G + 1)], band_tiles[G][:], sidx[:],
                channels=128, num_elems=256, num_idxs=NBAND)
            add_dep_helper(si.ins, gather_insts[-1].ins,
                           reason="batch scatter ucode after all gathers")

        # ---- rn = 1 / max(sqrt(n2), eps), [x, y] ------------------------
        rn1 = scr_p.tile([128, MFX], F32, name="rn1", tag="rn1")
        nc.vector.memset(rn1[:], 0.0)
        nc.scalar.activation(
            _ap(rn1, 0, [[MFX, 128], [16, 8], [1, 16]]),
            _ap(f2, 32, [[FX2, 128], [256, 8], [1, 16]]),
            ACTF.Sqrt)
        nc.vector.tensor_scalar(rn1[:, 0:128], rn1[:, 0:128], 1e-8, None,
                                op0=ALU.max)
        rnT = scr_p.tile([128, MFX], F32, name="rnT", tag="rnT")
        nc.vector.memset(rnT[:], 0.0)
        nc.vector.reciprocal(rnT[:, 0:128], rn1[:, 0:128])

        rsh = {0: rnT}
        for dx in (1, 2, -1, -2):
            rsh[dx] = make_shift("r", rnT, dx)

        pA = scr_p.tile([128, FX2], BF16, name="pA", tag="pA")
        pB = scr_p.tile([128, FX2], BF16, name="pB", tag="pB")
        nc.vector.memset(pA[:], 0.0)
        for dx, dy in slots():
            slot_op(pA, rnT, rsh[dx], dx, dy, ALU.mult)

        # ---- pointwise + fused reduction (A/B ping-pong) ----------------
        nc.vector.tensor_tensor(pB[:], f2[:], pA[:], op=ALU.mult)   # cos
        nc.vector.tensor_tensor(pA[:], pB[:], lab[:], op=ALU.subtract)  # d
        nc.vector.tensor_tensor(pB[:], pA[:], pA[:], op=ALU.mult)   # d^2
        nc.vector.scalar_tensor_tensor(
            pA[:], pB[:], 1.0, Wt[:], op0=ALU.mult, op1=ALU.mult,
            accum_out=R[:, 0:1])

        # ---- final scalar tail ------------------------------------------
        ps2 = psum_p.tile([128, 2048], F32, name="P", tag="P")
        nc.tensor.matmul(ps2[0:1, 0:4], ones_f[:, 0:1], R[:, 0:4],
                         start=True, stop=True)
        scal = scr_p.tile([1, 32], F32, name="scal", tag="scal")
        nc.scalar.copy(scal[0:1, 0:4], ps2[0:1, 0:4])
        # scal: 0=S, 1=cnt, 2=gtbsum | 4=include, 5=max(cnt,1), 6=1/max, 7=loss
        nc.vector.tensor_scalar(scal[0:1, 4:5], scal[0:1, 2:3], 0.0, None,
                                op0=ALU.is_gt)
        nc.vector.tensor_scalar(scal[0:1, 5:6], scal[0:1, 1:2], 1.0, None,
                                op0=ALU.max)
        nc.vector.reciprocal(scal[0:1, 6:7], scal[0:1, 5:6])
        nc.vector.tensor_tensor(scal[0:1, 7:8], scal[0:1, 0:1],
                                scal[0:1, 6:7], op=ALU.mult)
        nc.vector.tensor_tensor(scal[0:1, 7:8], scal[0:1, 7:8],
                                scal[0:1, 4:5], op=ALU.mult)
        nc.vector.tensor_scalar(scal[0:1, 7:8], scal[0:1, 7:8],
                                1.0 / 24.0, None, op0=ALU.mult)

        outt = scr_p.tile([1, 32], F32, name="outt", tag="outt")
        nc.vector.tensor_copy(outt[0:1, 0:1], scal[0:1, 7:8])
        nc.vector.tensor_copy(outt[0:1, 1:2], scal[0:1, 4:5])
        nc.sync.dma_start(out=out_d.ap(), in_=outt[0:1, 0:2])


_NC_CACHE = {}


def _make_idx():
    # gather: per core group g16=(x//16), band (r16, dy):
    #   pairs start (392*r16 + 128*dy + 16*g16)/2, 10 pairs
    gidx = np.zeros((128, NPAIR // 16), np.int16)
    for g16 in range(8):
        lo = 16 * g16
        unwrapped = np.zeros(NPAIR, np.int64)
        i = 0
        for r16 in range(16):
            for dy in range(3):
                start = 392 * r16 + 128 * dy + 16 * g16
                for j in range(10):
                    unwrapped[i] = start // 2 + j
                    i += 1
        for i in range(NPAIR):
            gidx[lo + i % 16, i // 16] = unwrapped[i]

    # scatter: bands elem m: pair i=m//2, o=m%2, band=i//10, jp=i%10,
    # j = 2*jp+o in [0,20); r16 = band//3, dy = band%3;
    # dx = j - (x%16) - 2; target 16*(5dy+dx+2) + r16 if |dx|<=2
    sidx = np.full((128, NBAND), -1, np.int16)
    for x in range(128):
        xm = x % 16
        for i in range(NPAIR):
            band, jp = divmod(i, 10)
            r16, dy = divmod(band, 3)
            for o in range(2):
                j = 2 * jp + o
                dx = j - xm - 2
                if -2 <= dx <= 2:
                    sidx[x, 2 * i + o] = 16 * (5 * dy + dx + 2) + r16
    return gidx, sidx


def get_nc():
    if "nc" not in _NC_CACHE:
        nc = bacc.Bacc("TRN2", target_bir_lowering=False, debug=False)
        build_kernel(nc)
        _NC_CACHE["nc"] = nc
    return _NC_CACHE["nc"]


def kernel(er_input, seg_label, gt_boundary_seg):
    er = np.ascontiguousarray(np.asarray(er_input, dtype=np.float32))
    seg = np.ascontiguousarray(np.asarray(seg_label, dtype=np.int32))
    gtb = np.ascontiguousarray(np.asarray(gt_boundary_seg, dtype=np.int32))
    assert er.shape == (B, C, H, W), er.shape

    nc = get_nc()
    from concourse.bass_utils import run_bass_kernel_spmd

    gidx, sidx = _make_idx()
    in_maps = [
        {"er": er[i], "seg": seg[i], "gtb": gtb[i], "gidx": gidx,
         "sidx": sidx}
        for i in range(B)
    ]
    res = run_bass_kernel_spmd(nc, in_maps, list(range(B)))
    outs = [res.results[i]["out"] for i in range(B)]
    loss_nums = np.array([o[0, 0] for o in outs], dtype=np.float64)
    incs = np.array([o[0, 1] for o in outs], dtype=np.float64)
    loss = loss_nums.sum() / max(incs.sum(), 1.0)
    return np.float32(loss)
